# revision 2
# baseline (speedup 1.0000x reference)
"""HakesIVF select_centers kernel for Trainium2 (Bass/Tile), 8-core data parallel.

Algorithm:
  assign[i] = argmin_j ||x_i - c_j||^2  ==  argmax_j (x_i . c_j - ||c_j||^2 / 2)
  out[i]    = centroids[assign[i]]

Per core (vecs sharded on N, 32768 rows each):
  - PE: scores via one K=65 matmul per 128-row tile.  lhsT = [vecsT; ones] tile
    (65 x 128), rhs = [centroidsT; -csq/2] (65 x 1024), psum tile (128 x 1024).
    The appended row folds the -||c||^2/2 bias into the matmul.
  - DVE: InstMax (top-8) + InstMaxIndex over each psum tile -> argmax index.
  - GPSIMD indirect DMA: gather centroid rows from DRAM by index.
  - Batched contiguous stores of gathered rows.

Inputs are transposed/augmented on the host (layout prep for sharding); all
FLOPs, reductions and the gather run on the NeuronCores.
"""

import numpy as np

N, NLIST, D = 262144, 1024, 64
NCORES = 8
NPC = N // NCORES          # 32768 rows per core
P = 128                    # rows per tile
NT = NPC // P              # 256 tiles per core
CHUNK = 16                 # tiles per load/store chunk
KA = D + 1                 # augmented contraction dim (64 dims + bias row)

_cached = {}


def build_nc(npc=NPC, nlist=NLIST, d=D, chunk=CHUNK, n_queues=4):
    """Build and compile the per-core Bass module. Same program on all cores."""
    from contextlib import ExitStack

    import concourse.tile as tile
    from concourse import bacc, bass, mybir

    ka = d + 1
    nt = npc // P
    assert nt % chunk == 0
    fp32 = mybir.dt.float32

    nc = bacc.Bacc(
        "TRN2",
        target_bir_lowering=False,
        debug=False,
        num_devices=NCORES,
        num_swdge_queues=n_queues,
    )
    vt = nc.dram_tensor("vt", [ka, npc], fp32, kind="ExternalInput").ap()
    ct = nc.dram_tensor("ct", [ka, nlist], fp32, kind="ExternalInput").ap()
    ctab = nc.dram_tensor("ctab", [nlist, d], fp32, kind="ExternalInput").ap()
    out = nc.dram_tensor("out", [npc, d], fp32, kind="ExternalOutput").ap()

    with tile.TileContext(nc) as tc, ExitStack() as ctx:
        const_pool = ctx.enter_context(tc.tile_pool(name="const", bufs=1))
        vchunk_pool = ctx.enter_context(tc.tile_pool(name="vchunk", bufs=3))
        psum_pool = ctx.enter_context(tc.tile_pool(name="psum", bufs=4, space="PSUM"))
        small_pool = ctx.enter_context(tc.tile_pool(name="small", bufs=4))
        gout_pool = ctx.enter_context(tc.tile_pool(name="gout", bufs=3))

        ct_sb = const_pool.tile([ka, nlist], fp32)
        nc.sync.dma_start(ct_sb[:], ct[:])

        for c in range(nt // chunk):
            # one big strided load: [ka, chunk*P] slab of transposed vecs
            vch = vchunk_pool.tile([ka, chunk * P], fp32)
            nc.sync.dma_start(vch[:], vt[:, c * chunk * P : (c + 1) * chunk * P])
            gout = gout_pool.tile([P, chunk * d], fp32)
            for g in range(chunk):
                w = vch[:, g * P : (g + 1) * P]
                ps = psum_pool.tile([P, nlist], fp32)
                for h in range(nlist // 512):
                    nc.tensor.matmul(
                        ps[:, h * 512 : (h + 1) * 512],
                        lhsT=w,
                        rhs=ct_sb[:, h * 512 : (h + 1) * 512],
                        start=True,
                        stop=True,
                    )
                m8 = small_pool.tile([P, 8], fp32, tag="m8")
                nc.vector.max(m8[:], ps[:])
                idx8 = small_pool.tile([P, 8], mybir.dt.uint32, tag="idx8")
                nc.vector.max_index(idx8[:], m8[:], ps[:])
                nc.gpsimd.indirect_dma_start(
                    out=gout[:, g * d : (g + 1) * d],
                    out_offset=None,
                    in_=ctab[:],
                    in_offset=bass.IndirectOffsetOnAxis(ap=idx8[:, 0:1], axis=0),
                )
            # store chunk: row (c*chunk + g)*P + p  <- gout[p, g*d:(g+1)*d]
            dst = out[c * chunk * P : (c + 1) * chunk * P, :].rearrange(
                "(g p) d -> p g d", p=P
            )
            nc.sync.dma_start(dst, gout[:].rearrange("p (g d) -> p g d", d=d))

    nc.compile()
    return nc


def _prep_inputs(vecs, centroids):
    """Host-side shard + layout prep. Returns per-core input maps."""
    vecs = np.ascontiguousarray(np.asarray(vecs), dtype=np.float32)
    cents = np.ascontiguousarray(np.asarray(centroids), dtype=np.float32)
    csq = np.sum(cents * cents, axis=1, dtype=np.float32)

    ct_aug = np.empty((KA, NLIST), dtype=np.float32)
    ct_aug[:D] = cents.T
    ct_aug[D] = -0.5 * csq

    in_maps = []
    for c in range(NCORES):
        sl = vecs[c * NPC : (c + 1) * NPC]
        vt = np.empty((KA, NPC), dtype=np.float32)
        vt[:D] = sl.T
        vt[D] = 1.0
        in_maps.append({"vt": vt, "ct": ct_aug, "ctab": cents})
    return in_maps


def kernel(vecs, centroids):
    from concourse.bass_utils import run_bass_kernel_spmd

    if "nc" not in _cached:
        _cached["nc"] = build_nc()
    nc = _cached["nc"]

    in_maps = _prep_inputs(vecs, centroids)
    res = run_bass_kernel_spmd(nc, in_maps, core_ids=list(range(NCORES)))
    outs = [res.results[c]["out"] for c in range(NCORES)]
    return np.concatenate(outs, axis=0)


# revision 3
# speedup vs baseline: 1.4450x; 1.4450x over previous
"""HakesIVF select_centers kernel for Trainium2 (Bass/Tile), 8-core data parallel.

Algorithm:
  assign[i] = argmin_j ||x_i - c_j||^2  ==  argmax_j (x_i . c_j - ||c_j||^2 / 2)
  out[i]    = centroids[assign[i]]

Per core (vecs sharded on N, 32768 rows each):
  - PE: scores for each 128-row tile into a (128 x 1024) fp32 psum tile.
    fp32 matmuls run at quarter rate on trn2, so the fp32 dot products are
    computed with an exact fp16 two-term split instead:
      x = h1 + h2,  c = g1 + g2   (h2,g2 = fp16 residuals)
      x.c ~= h1.g1 + h1.g2 + h2.g1   (h2.g2 <= ~3e-6, below fp32 noise)
    fp16 x fp16 products are exact in fp32, PSUM accumulates in fp32, so
    scores stay in the fp32-accuracy class (verified: 0 argmax flips vs
    direct fp32 on the target data).
      mm1: K=66  lhsT=[h1; 1; 1]    rhs=[g1; b1; b2]   (b = -|c|^2/2 split)
      mm2: K=128 lhsT=[h1; h2]      rhs=[g2; g1]       (accumulate)
  - DVE: MAX8 (top-8) + FIND_INDEX8 over each psum tile -> argmax index.
  - GPSIMD indirect DMA: gather centroid rows from DRAM by index.
  - Batched contiguous stores of gathered rows.

Inputs are transposed/split on the host (layout prep for sharding); all
FLOPs, reductions and the gather run on the NeuronCores.
"""

import numpy as np

N, NLIST, D = 262144, 1024, 64
NCORES = 8
NPC = N // NCORES          # 32768 rows per core
P = 128                    # rows per tile
NT = NPC // P              # 256 tiles per core
CHUNK = 16                 # tiles per load/store chunk
K1 = D + 2                 # 66:  h1 rows + two ones rows (bias halves)
K2 = 2 * D                 # 128: h1 rows + h2 rows

_cached = {}


def build_nc(npc=NPC, nlist=NLIST, d=D, chunk=CHUNK, n_queues=4):
    """Build and compile the per-core Bass module. Same program on all cores."""
    from contextlib import ExitStack

    import concourse.tile as tile
    from concourse import bacc, bass, mybir

    k1, k2 = d + 2, 2 * d
    nt = npc // P
    assert nt % chunk == 0
    fp32 = mybir.dt.float32
    fp16 = mybir.dt.float16

    nc = bacc.Bacc(
        "TRN2",
        target_bir_lowering=False,
        debug=False,
        num_devices=NCORES,
        num_swdge_queues=n_queues,
    )
    vt1 = nc.dram_tensor("vt1", [k1, npc], fp16, kind="ExternalInput").ap()
    vt2 = nc.dram_tensor("vt2", [k2, npc], fp16, kind="ExternalInput").ap()
    ct1 = nc.dram_tensor("ct1", [k1, nlist], fp16, kind="ExternalInput").ap()
    ct2 = nc.dram_tensor("ct2", [k2, nlist], fp16, kind="ExternalInput").ap()
    ctab = nc.dram_tensor("ctab", [nlist, d], fp32, kind="ExternalInput").ap()
    out = nc.dram_tensor("out", [npc, d], fp32, kind="ExternalOutput").ap()

    with tile.TileContext(nc) as tc, ExitStack() as ctx:
        const_pool = ctx.enter_context(tc.tile_pool(name="const", bufs=1))
        vchunk_pool = ctx.enter_context(tc.tile_pool(name="vchunk", bufs=3))
        psum_pool = ctx.enter_context(tc.tile_pool(name="psum", bufs=4, space="PSUM"))
        small_pool = ctx.enter_context(tc.tile_pool(name="small", bufs=4))
        gout_pool = ctx.enter_context(tc.tile_pool(name="gout", bufs=3))

        ct1_sb = const_pool.tile([k1, nlist], fp16)
        nc.sync.dma_start(ct1_sb[:], ct1[:])
        ct2_sb = const_pool.tile([k2, nlist], fp16)
        nc.sync.dma_start(ct2_sb[:], ct2[:])

        for c in range(nt // chunk):
            # strided slab loads of the transposed split vecs
            vch1 = vchunk_pool.tile([k1, chunk * P], fp16, tag="vch1")
            nc.sync.dma_start(vch1[:], vt1[:, c * chunk * P : (c + 1) * chunk * P])
            vch2 = vchunk_pool.tile([k2, chunk * P], fp16, tag="vch2")
            nc.sync.dma_start(vch2[:], vt2[:, c * chunk * P : (c + 1) * chunk * P])
            gout = gout_pool.tile([P, chunk * d], fp32)
            for g in range(chunk):
                w1 = vch1[:, g * P : (g + 1) * P]
                w2 = vch2[:, g * P : (g + 1) * P]
                ps = psum_pool.tile([P, nlist], fp32)
                for h in range(nlist // 512):
                    sl = slice(h * 512, (h + 1) * 512)
                    nc.tensor.matmul(
                        ps[:, sl], lhsT=w1, rhs=ct1_sb[:, sl], start=True, stop=False
                    )
                    nc.tensor.matmul(
                        ps[:, sl], lhsT=w2, rhs=ct2_sb[:, sl], start=False, stop=True
                    )
                m8 = small_pool.tile([P, 8], fp32, tag="m8")
                nc.vector.max(m8[:], ps[:])
                idx8 = small_pool.tile([P, 8], mybir.dt.uint32, tag="idx8")
                nc.vector.max_index(idx8[:], m8[:], ps[:])
                nc.gpsimd.indirect_dma_start(
                    out=gout[:, g * d : (g + 1) * d],
                    out_offset=None,
                    in_=ctab[:],
                    in_offset=bass.IndirectOffsetOnAxis(ap=idx8[:, 0:1], axis=0),
                )
            # store chunk: row (c*chunk + g)*P + p  <- gout[p, g*d:(g+1)*d]
            dst = out[c * chunk * P : (c + 1) * chunk * P, :].rearrange(
                "(g p) d -> p g d", p=P
            )
            nc.sync.dma_start(dst, gout[:].rearrange("p (g d) -> p g d", d=d))

    nc.compile()
    return nc


def _split16(a):
    hi = a.astype(np.float16)
    lo = (a - hi.astype(np.float32)).astype(np.float16)
    return hi, lo


def _prep_inputs(vecs, centroids):
    """Host-side shard + layout prep. Returns per-core input maps."""
    vecs = np.ascontiguousarray(np.asarray(vecs), dtype=np.float32)
    cents = np.ascontiguousarray(np.asarray(centroids), dtype=np.float32)
    csq = np.sum(cents * cents, axis=1, dtype=np.float32)
    b1, b2 = _split16(-0.5 * csq)
    g1, g2 = _split16(cents)

    ct1 = np.empty((K1, NLIST), dtype=np.float16)
    ct1[:D] = g1.T
    ct1[D] = b1
    ct1[D + 1] = b2
    ct2 = np.empty((K2, NLIST), dtype=np.float16)
    ct2[:D] = g2.T
    ct2[D:] = g1.T

    in_maps = []
    for c in range(NCORES):
        sl = vecs[c * NPC : (c + 1) * NPC]
        h1, h2 = _split16(sl)
        vt1 = np.empty((K1, NPC), dtype=np.float16)
        vt1[:D] = h1.T
        vt1[D:] = 1.0
        vt2 = np.empty((K2, NPC), dtype=np.float16)
        vt2[:D] = h1.T
        vt2[D:] = h2.T
        in_maps.append({"vt1": vt1, "vt2": vt2, "ct1": ct1, "ct2": ct2, "ctab": cents})
    return in_maps


def kernel(vecs, centroids):
    from concourse.bass_utils import run_bass_kernel_spmd

    if "nc" not in _cached:
        _cached["nc"] = build_nc()
    nc = _cached["nc"]

    in_maps = _prep_inputs(vecs, centroids)
    res = run_bass_kernel_spmd(nc, in_maps, core_ids=list(range(NCORES)))
    outs = [res.results[c]["out"] for c in range(NCORES)]
    return np.concatenate(outs, axis=0)


# revision 7
# speedup vs baseline: 2.2656x; 1.5678x over previous
"""HakesIVF select_centers kernel for Trainium2 (Bass/Tile), 8-core data parallel.

Algorithm:
  assign[i] = argmin_j ||x_i - c_j||^2  ==  argmax_j (x_i . c_j - ||c_j||^2 / 2)
  out[i]    = centroids[assign[i]]

Per core (vecs sharded on N, 32768 rows each):
  - PE: scores for each 128-row tile into a (128 x 1024) fp32 psum tile.
    fp32 matmuls run at quarter rate on trn2, so the fp32 dot products are
    computed with an exact fp16 two-term split instead:
      x = h1 + h2,  c = g1 + g2   (h2,g2 = fp16 residuals)
      x.c ~= h1.g1 + h1.g2 + h2.g1   (h2.g2 <= ~3e-6, below fp32 noise)
    fp16 x fp16 products are exact in fp32, PSUM accumulates in fp32, so
    scores stay in the fp32-accuracy class (verified: 0 argmax flips vs
    direct fp32 on the target data).
      mm1: K=66  lhsT=[h1; 1; 1]    rhs=[g1; b1; b2]   (b = -|c|^2/2 split)
      mm2: K=128 lhsT=[h1; h2]      rhs=[g2; g1]       (accumulate)
  - DVE: MAX8 (top-8) + FIND_INDEX8 over each psum tile -> argmax index.
  - GPSIMD indirect DMA: gather centroid rows from DRAM by index.
  - Batched contiguous stores of gathered rows.

Inputs are transposed/split on the host (layout prep for sharding); all
FLOPs, reductions and the gather run on the NeuronCores.
"""

import numpy as np

N, NLIST, D = 262144, 1024, 64
NCORES = 8
NPC = N // NCORES          # 32768 rows per core
P = 128                    # rows per tile
NT = NPC // P              # 256 tiles per core
CHUNK = 16                 # tiles per load/store chunk
K1 = D + 2                 # 66:  h1 rows + two ones rows (bias halves)
K2 = 2 * D                 # 128: h1 rows + h2 rows

_cached = {}


def _register_argmax_op():
    """Register a custom DVE op fusing max + argmax into one pass.

    out[p,k]    = k if in0[p,k] == running_max(in0[p,:k+1]) else -1
    accum_out[p] = max_k out[p,k]
                 = last position where the running max updates
                 = argmax (exact when the row max is unique; ties pick the
                   last duplicate -- prob ~0 for random fp32 data, and the
                   target dataset has no exactly-tied row maxima).
    One 1x DVE pass replaces MAX8 + FIND_INDEX8 (two passes).
    """
    import numpy as np_

    from concourse import dve_ops
    from concourse.dve_spec import AluOp, Idx, One, Spec, Src0, Zero, eq, lower, scan, select
    from concourse.dve_uop import DveOpSpec

    NAME = "ARGMAX_SCAN_ANT"
    for op in dve_ops.OPS:
        if op.name == NAME:
            return op

    def _ref(in0, in1, s0, s1, imm2):
        x = np_.asarray(in0, dtype=np_.float32)
        r = np_.maximum.accumulate(x, axis=-1)
        idx = np_.arange(x.shape[-1], dtype=np_.float32)
        body = np_.where(x == r, idx, -1.0).astype(np_.float32)
        acc = body.max(axis=-1, keepdims=True)
        return body, acc

    spec = Spec(
        body=select(eq(Src0, scan(AluOp.MAX, Src0)), Idx, Zero - One),
        accum=AluOp.MAX,
        reference=_ref,
    )
    row = dve_ops._CUSTOM_DVE_ROW_BASE + len(dve_ops.OPS)
    assert row < 0x20
    uops_sha = {}
    for ver in ("v3", "v4"):
        compiled = DveOpSpec(name=NAME, opcode=row, uops=lower(spec, ver=ver), rd1_en=False)
        uops_sha[ver] = compiled.sha(ver)
    op = dve_ops.DveOp(NAME, spec, subdim=False, uops_sha=uops_sha)
    dve_ops.OPS.append(op)
    dve_ops.CUSTOM_DVE_SPECS[NAME] = spec
    dve_ops._SUB_OPCODE_FOR_NAME[NAME] = row
    return op


def build_nc(npc=NPC, nlist=NLIST, d=D, chunk=CHUNK, n_queues=4):
    """Build and compile the per-core Bass module. Same program on all cores."""
    from contextlib import ExitStack

    import concourse.tile as tile
    from concourse import bacc, bass, mybir

    k1, k2 = d + 2, 2 * d
    nt = npc // P
    assert nt % chunk == 0
    fp32 = mybir.dt.float32
    fp16 = mybir.dt.float16

    argmax_op = _register_argmax_op()

    nc = bacc.Bacc(
        "TRN2",
        target_bir_lowering=False,
        debug=False,
        num_devices=NCORES,
        num_swdge_queues=n_queues,
    )
    vt1 = nc.dram_tensor("vt1", [k1, npc], fp16, kind="ExternalInput").ap()
    vt2 = nc.dram_tensor("vt2", [k2, npc], fp16, kind="ExternalInput").ap()
    ct1 = nc.dram_tensor("ct1", [k1, nlist], fp16, kind="ExternalInput").ap()
    ct2 = nc.dram_tensor("ct2", [k2, nlist], fp16, kind="ExternalInput").ap()
    ctab = nc.dram_tensor("ctab", [nlist, d], fp32, kind="ExternalInput").ap()
    out = nc.dram_tensor("out", [npc, d], fp32, kind="ExternalOutput").ap()

    with tile.TileContext(nc) as tc, ExitStack() as ctx:
        const_pool = ctx.enter_context(tc.tile_pool(name="const", bufs=1))
        vchunk_pool = ctx.enter_context(tc.tile_pool(name="vchunk", bufs=3))
        psum_pool = ctx.enter_context(tc.tile_pool(name="psum", bufs=4, space="PSUM"))
        small_pool = ctx.enter_context(tc.tile_pool(name="small", bufs=4))
        scratch_pool = ctx.enter_context(tc.tile_pool(name="scratch", bufs=2))
        gout_pool = ctx.enter_context(tc.tile_pool(name="gout", bufs=3))

        ct1_sb = const_pool.tile([k1, nlist], fp16)
        nc.sync.dma_start(ct1_sb[:], ct1[:])
        ct2_sb = const_pool.tile([k2, nlist], fp16)
        nc.sync.dma_start(ct2_sb[:], ct2[:])

        for c in range(nt // chunk):
            # strided slab loads of the transposed split vecs
            vch1 = vchunk_pool.tile([k1, chunk * P], fp16, tag="vch1")
            nc.sync.dma_start(vch1[:], vt1[:, c * chunk * P : (c + 1) * chunk * P])
            vch2 = vchunk_pool.tile([k2, chunk * P], fp16, tag="vch2")
            nc.sync.dma_start(vch2[:], vt2[:, c * chunk * P : (c + 1) * chunk * P])
            gout = gout_pool.tile([P, chunk * d], fp32)
            for g in range(chunk):
                w1 = vch1[:, g * P : (g + 1) * P]
                w2 = vch2[:, g * P : (g + 1) * P]
                ps = psum_pool.tile([P, nlist], fp32)
                # same-weight matmuls adjacent (w1,w1,w2,w2) to cut LDWEIGHTS
                for h in range(nlist // 512):
                    sl = slice(h * 512, (h + 1) * 512)
                    nc.tensor.matmul(
                        ps[:, sl], lhsT=w1, rhs=ct1_sb[:, sl], start=True, stop=False
                    )
                for h in range(nlist // 512):
                    sl = slice(h * 512, (h + 1) * 512)
                    nc.tensor.matmul(
                        ps[:, sl], lhsT=w2, rhs=ct2_sb[:, sl], start=False, stop=True
                    )
                scr = scratch_pool.tile([P, nlist], fp32, tag="scr")
                idxf = small_pool.tile([P, 1], fp32, tag="idxf")
                nc.vector._custom_dve(
                    argmax_op, out=scr[:], in0=ps[:], accum_out=idxf[:]
                )
                idxu = small_pool.tile([P, 1], mybir.dt.uint32, tag="idxu")
                nc.vector.tensor_copy(idxu[:], idxf[:])
                nc.gpsimd.indirect_dma_start(
                    out=gout[:, g * d : (g + 1) * d],
                    out_offset=None,
                    in_=ctab[:],
                    in_offset=bass.IndirectOffsetOnAxis(ap=idxu[:, 0:1], axis=0),
                )
            # store chunk: row (c*chunk + g)*P + p  <- gout[p, g*d:(g+1)*d]
            dst = out[c * chunk * P : (c + 1) * chunk * P, :].rearrange(
                "(g p) d -> p g d", p=P
            )
            nc.sync.dma_start(dst, gout[:].rearrange("p (g d) -> p g d", d=d))

    nc.compile()
    return nc


def _split16(a):
    hi = a.astype(np.float16)
    lo = (a - hi.astype(np.float32)).astype(np.float16)
    return hi, lo


def _prep_inputs(vecs, centroids):
    """Host-side shard + layout prep. Returns per-core input maps."""
    vecs = np.ascontiguousarray(np.asarray(vecs), dtype=np.float32)
    cents = np.ascontiguousarray(np.asarray(centroids), dtype=np.float32)
    csq = np.sum(cents * cents, axis=1, dtype=np.float32)
    b1, b2 = _split16(-0.5 * csq)
    g1, g2 = _split16(cents)

    ct1 = np.empty((K1, NLIST), dtype=np.float16)
    ct1[:D] = g1.T
    ct1[D] = b1
    ct1[D + 1] = b2
    ct2 = np.empty((K2, NLIST), dtype=np.float16)
    ct2[:D] = g2.T
    ct2[D:] = g1.T

    in_maps = []
    for c in range(NCORES):
        sl = vecs[c * NPC : (c + 1) * NPC]
        h1, h2 = _split16(sl)
        vt1 = np.empty((K1, NPC), dtype=np.float16)
        vt1[:D] = h1.T
        vt1[D:] = 1.0
        vt2 = np.empty((K2, NPC), dtype=np.float16)
        vt2[:D] = h1.T
        vt2[D:] = h2.T
        in_maps.append({"vt1": vt1, "vt2": vt2, "ct1": ct1, "ct2": ct2, "ctab": cents})
    return in_maps


def kernel(vecs, centroids):
    from concourse.bass_utils import run_bass_kernel_spmd

    if "nc" not in _cached:
        _cached["nc"] = build_nc()
    nc = _cached["nc"]

    in_maps = _prep_inputs(vecs, centroids)
    res = run_bass_kernel_spmd(nc, in_maps, core_ids=list(range(NCORES)))
    outs = [res.results[c]["out"] for c in range(NCORES)]
    return np.concatenate(outs, axis=0)


# revision 10
# speedup vs baseline: 2.3056x; 1.0177x over previous
"""HakesIVF select_centers kernel for Trainium2 (Bass/Tile), 8-core data parallel.

Algorithm:
  assign[i] = argmin_j ||x_i - c_j||^2  ==  argmax_j (x_i . c_j - ||c_j||^2 / 2)
  out[i]    = centroids[assign[i]]

Per core (vecs sharded on N, 32768 rows each):
  - PE: scores for each 128-row tile into a (128 x 1024) fp32 psum tile.
    fp32 matmuls run at quarter rate on trn2, so the fp32 dot products are
    computed with an exact fp16 two-term split instead:
      x = h1 + h2,  c = g1 + g2   (h2,g2 = fp16 residuals)
      x.c ~= h1.g1 + h1.g2 + h2.g1   (h2.g2 <= ~3e-6, below fp32 noise)
    fp16 x fp16 products are exact in fp32, PSUM accumulates in fp32, so
    scores stay in the fp32-accuracy class (verified: 0 argmax flips vs
    direct fp32 on the target data).
      mm1: K=66  lhsT=[h1; 1; 1]    rhs=[g1; b1; b2]   (b = -|c|^2/2 split)
      mm2: K=128 lhsT=[h1; h2]      rhs=[g2; g1]       (accumulate)
  - DVE: MAX8 (top-8) + FIND_INDEX8 over each psum tile -> argmax index.
  - GPSIMD indirect DMA: gather centroid rows from DRAM by index.
  - Batched contiguous stores of gathered rows.

Inputs are transposed/split on the host (layout prep for sharding); all
FLOPs, reductions and the gather run on the NeuronCores.
"""

import numpy as np

N, NLIST, D = 262144, 1024, 64
NCORES = 8
NPC = N // NCORES          # 32768 rows per core
P = 128                    # rows per tile
NT = NPC // P              # 256 tiles per core
CHUNK = 16                 # tiles per load/store chunk
K1 = D + 2                 # 66:  h1 rows + two ones rows (bias halves)
K2 = 2 * D                 # 128: h1 rows + h2 rows

_cached = {}


def _register_argmax_op():
    """Register a custom DVE op fusing max + argmax into one pass.

    out[p,k]    = k if in0[p,k] == running_max(in0[p,:k+1]) else -1
    accum_out[p] = max_k out[p,k]
                 = last position where the running max updates
                 = argmax (exact when the row max is unique; ties pick the
                   last duplicate -- prob ~0 for random fp32 data, and the
                   target dataset has no exactly-tied row maxima).
    One 1x DVE pass replaces MAX8 + FIND_INDEX8 (two passes).
    """
    import numpy as np_

    from concourse import dve_ops
    from concourse.dve_spec import AluOp, Idx, One, Spec, Src0, Zero, eq, lower, scan, select
    from concourse.dve_uop import DveOpSpec

    NAME = "ARGMAX_SCAN_ANT"
    for op in dve_ops.OPS:
        if op.name == NAME:
            return op

    def _ref(in0, in1, s0, s1, imm2):
        x = np_.asarray(in0, dtype=np_.float32)
        r = np_.maximum.accumulate(x, axis=-1)
        idx = np_.arange(x.shape[-1], dtype=np_.float32)
        body = np_.where(x == r, idx, -1.0).astype(np_.float32)
        acc = body.max(axis=-1, keepdims=True)
        return body, acc

    spec = Spec(
        body=select(eq(Src0, scan(AluOp.MAX, Src0)), Idx, Zero - One),
        accum=AluOp.MAX,
        reference=_ref,
    )
    row = dve_ops._CUSTOM_DVE_ROW_BASE + len(dve_ops.OPS)
    assert row < 0x20
    uops_sha = {}
    for ver in ("v3", "v4"):
        compiled = DveOpSpec(name=NAME, opcode=row, uops=lower(spec, ver=ver), rd1_en=False)
        uops_sha[ver] = compiled.sha(ver)
    op = dve_ops.DveOp(NAME, spec, subdim=False, uops_sha=uops_sha)
    dve_ops.OPS.append(op)
    dve_ops.CUSTOM_DVE_SPECS[NAME] = spec
    dve_ops._SUB_OPCODE_FOR_NAME[NAME] = row
    return op


def build_nc(npc=NPC, nlist=NLIST, d=D, chunk=CHUNK, n_queues=4):
    """Build and compile the per-core Bass module. Same program on all cores."""
    from contextlib import ExitStack

    import concourse.tile as tile
    from concourse import bacc, bass, mybir

    k1, k2 = d + 2, 2 * d
    nt = npc // P
    assert nt % chunk == 0
    fp32 = mybir.dt.float32
    fp16 = mybir.dt.float16

    argmax_op = _register_argmax_op()

    nc = bacc.Bacc(
        "TRN2",
        target_bir_lowering=False,
        debug=False,
        num_devices=NCORES,
        num_swdge_queues=n_queues,
    )
    vt1 = nc.dram_tensor("vt1", [k1, npc], fp16, kind="ExternalInput").ap()
    vt2 = nc.dram_tensor("vt2", [k2, npc], fp16, kind="ExternalInput").ap()
    ct1 = nc.dram_tensor("ct1", [k1, nlist], fp16, kind="ExternalInput").ap()
    ct2 = nc.dram_tensor("ct2", [k2, nlist], fp16, kind="ExternalInput").ap()
    ctab = nc.dram_tensor("ctab", [nlist, d], fp32, kind="ExternalInput").ap()
    out = nc.dram_tensor("out", [npc, d], fp32, kind="ExternalOutput").ap()

    with tile.TileContext(nc) as tc, ExitStack() as ctx:
        const_pool = ctx.enter_context(tc.tile_pool(name="const", bufs=1))
        vchunk_pool = ctx.enter_context(tc.tile_pool(name="vchunk", bufs=3))
        psum_pool = ctx.enter_context(tc.tile_pool(name="psum", bufs=4, space="PSUM"))
        small_pool = ctx.enter_context(tc.tile_pool(name="small", bufs=4))
        scratch_pool = ctx.enter_context(tc.tile_pool(name="scratch", bufs=2))
        gout_pool = ctx.enter_context(tc.tile_pool(name="gout", bufs=3))

        ct1_sb = const_pool.tile([k1, nlist], fp16)
        nc.sync.dma_start(ct1_sb[:], ct1[:])
        ct2_sb = const_pool.tile([k2, nlist], fp16)
        nc.sync.dma_start(ct2_sb[:], ct2[:])

        for c in range(nt // chunk):
            # strided slab loads of the transposed split vecs
            vch1 = vchunk_pool.tile([k1, chunk * P], fp16, tag="vch1")
            nc.sync.dma_start(vch1[:], vt1[:, c * chunk * P : (c + 1) * chunk * P])
            vch2 = vchunk_pool.tile([k2, chunk * P], fp16, tag="vch2")
            nc.sync.dma_start(vch2[:], vt2[:, c * chunk * P : (c + 1) * chunk * P])
            gout = gout_pool.tile([P, chunk * d], fp32)
            # process tiles in groups of gb: dense 4-tile matmul bursts keep
            # the PE busy past the HAM warmup window; one index-cast per group.
            # (HW SWDGE honors only one dynamic offset per partition, so the
            # gathers stay per-tile.)
            gb = min(4, chunk)
            for g0 in range(0, chunk, gb):
                pss = []
                for g in range(g0, g0 + gb):
                    w1 = vch1[:, g * P : (g + 1) * P]
                    w2 = vch2[:, g * P : (g + 1) * P]
                    ps = psum_pool.tile([P, nlist], fp32)
                    # same-weight matmuls adjacent (w1,w1,w2,w2)
                    for h in range(nlist // 512):
                        sl = slice(h * 512, (h + 1) * 512)
                        nc.tensor.matmul(
                            ps[:, sl], lhsT=w1, rhs=ct1_sb[:, sl],
                            start=True, stop=False,
                        )
                    for h in range(nlist // 512):
                        sl = slice(h * 512, (h + 1) * 512)
                        nc.tensor.matmul(
                            ps[:, sl], lhsT=w2, rhs=ct2_sb[:, sl],
                            start=False, stop=True,
                        )
                    pss.append(ps)
                idxf = small_pool.tile([P, gb], fp32, tag="idxf")
                for i, ps in enumerate(pss):
                    scr = scratch_pool.tile([P, nlist], fp32, tag="scr")
                    nc.vector._custom_dve(
                        argmax_op, out=scr[:], in0=ps[:],
                        accum_out=idxf[:, i : i + 1],
                    )
                idxu = small_pool.tile([P, gb], mybir.dt.uint32, tag="idxu")
                nc.vector.tensor_copy(idxu[:], idxf[:])
                for i in range(gb):
                    g = g0 + i
                    nc.gpsimd.indirect_dma_start(
                        out=gout[:, g * d : (g + 1) * d],
                        out_offset=None,
                        in_=ctab[:],
                        in_offset=bass.IndirectOffsetOnAxis(ap=idxu[:, i : i + 1], axis=0),
                    )
            # store chunk: row (c*chunk + g)*P + p  <- gout[p, g*d:(g+1)*d]
            dst = out[c * chunk * P : (c + 1) * chunk * P, :].rearrange(
                "(g p) d -> p g d", p=P
            )
            nc.sync.dma_start(dst, gout[:].rearrange("p (g d) -> p g d", d=d))

    nc.compile()
    return nc


def _split16(a):
    hi = a.astype(np.float16)
    lo = (a - hi.astype(np.float32)).astype(np.float16)
    return hi, lo


def _prep_inputs(vecs, centroids):
    """Host-side shard + layout prep. Returns per-core input maps."""
    vecs = np.ascontiguousarray(np.asarray(vecs), dtype=np.float32)
    cents = np.ascontiguousarray(np.asarray(centroids), dtype=np.float32)
    csq = np.sum(cents * cents, axis=1, dtype=np.float32)
    b1, b2 = _split16(-0.5 * csq)
    g1, g2 = _split16(cents)

    ct1 = np.empty((K1, NLIST), dtype=np.float16)
    ct1[:D] = g1.T
    ct1[D] = b1
    ct1[D + 1] = b2
    ct2 = np.empty((K2, NLIST), dtype=np.float16)
    ct2[:D] = g2.T
    ct2[D:] = g1.T

    in_maps = []
    for c in range(NCORES):
        sl = vecs[c * NPC : (c + 1) * NPC]
        h1, h2 = _split16(sl)
        vt1 = np.empty((K1, NPC), dtype=np.float16)
        vt1[:D] = h1.T
        vt1[D:] = 1.0
        vt2 = np.empty((K2, NPC), dtype=np.float16)
        vt2[:D] = h1.T
        vt2[D:] = h2.T
        in_maps.append({"vt1": vt1, "vt2": vt2, "ct1": ct1, "ct2": ct2, "ctab": cents})
    return in_maps


def kernel(vecs, centroids):
    from concourse.bass_utils import run_bass_kernel_spmd

    if "nc" not in _cached:
        _cached["nc"] = build_nc()
    nc = _cached["nc"]

    in_maps = _prep_inputs(vecs, centroids)
    res = run_bass_kernel_spmd(nc, in_maps, core_ids=list(range(NCORES)))
    outs = [res.results[c]["out"] for c in range(NCORES)]
    return np.concatenate(outs, axis=0)


# revision 13
# speedup vs baseline: 2.3267x; 1.0092x over previous
"""HakesIVF select_centers kernel for Trainium2 (Bass/Tile), 8-core data parallel.

Algorithm:
  assign[i] = argmin_j ||x_i - c_j||^2  ==  argmax_j (x_i . c_j - ||c_j||^2 / 2)
  out[i]    = centroids[assign[i]]

Per core (vecs sharded on N, 32768 rows each):
  - PE: scores for each 128-row tile into a (128 x 1024) fp32 psum tile.
    fp32 matmuls run at quarter rate on trn2, so the fp32 dot products are
    computed with an exact fp16 two-term split instead:
      x = h1 + h2,  c = g1 + g2   (h2,g2 = fp16 residuals)
      x.c ~= h1.g1 + h1.g2 + h2.g1   (h2.g2 <= ~3e-6, below fp32 noise)
    fp16 x fp16 products are exact in fp32, PSUM accumulates in fp32, so
    scores stay in the fp32-accuracy class (verified: 0 argmax flips vs
    direct fp32 on the target data).
      mm1: K=66  lhsT=[h1; 1; 1]    rhs=[g1; b1; b2]   (b = -|c|^2/2 split)
      mm2: K=128 lhsT=[h1; h2]      rhs=[g2; g1]       (accumulate)
  - DVE: MAX8 (top-8) + FIND_INDEX8 over each psum tile -> argmax index.
  - GPSIMD indirect DMA: gather centroid rows from DRAM by index.
  - Batched contiguous stores of gathered rows.

Inputs are transposed/split on the host (layout prep for sharding); all
FLOPs, reductions and the gather run on the NeuronCores.
"""

import numpy as np

N, NLIST, D = 262144, 1024, 64
NCORES = 8
NPC = N // NCORES          # 32768 rows per core
P = 128                    # rows per tile
NT = NPC // P              # 256 tiles per core
CHUNK = 16                 # tiles per load/store chunk
K1 = D + 2                 # 66:  h1 rows + two ones rows (bias halves)
K2 = 2 * D                 # 128: h1 rows + h2 rows

_cached = {}


def _register_argmax_op():
    """Register a custom DVE op fusing max + argmax into one pass.

    out[p,k]    = k if in0[p,k] == running_max(in0[p,:k+1]) else -1
    accum_out[p] = max_k out[p,k]
                 = last position where the running max updates
                 = argmax (exact when the row max is unique; ties pick the
                   last duplicate -- prob ~0 for random fp32 data, and the
                   target dataset has no exactly-tied row maxima).
    One 1x DVE pass replaces MAX8 + FIND_INDEX8 (two passes).
    """
    import numpy as np_

    from concourse import dve_ops
    from concourse.dve_spec import AluOp, Idx, One, Spec, Src0, Zero, eq, lower, scan, select
    from concourse.dve_uop import DveOpSpec

    NAME = "ARGMAX_SCAN_ANT"
    for op in dve_ops.OPS:
        if op.name == NAME:
            return op

    def _ref(in0, in1, s0, s1, imm2):
        x = np_.asarray(in0, dtype=np_.float32)
        r = np_.maximum.accumulate(x, axis=-1)
        idx = np_.arange(x.shape[-1], dtype=np_.float32)
        body = np_.where(x == r, idx, -1.0).astype(np_.float32)
        acc = body.max(axis=-1, keepdims=True)
        return body, acc

    spec = Spec(
        body=select(eq(Src0, scan(AluOp.MAX, Src0)), Idx, Zero - One),
        accum=AluOp.MAX,
        reference=_ref,
    )
    row = dve_ops._CUSTOM_DVE_ROW_BASE + len(dve_ops.OPS)
    assert row < 0x20
    uops_sha = {}
    for ver in ("v3", "v4"):
        compiled = DveOpSpec(name=NAME, opcode=row, uops=lower(spec, ver=ver), rd1_en=False)
        uops_sha[ver] = compiled.sha(ver)
    op = dve_ops.DveOp(NAME, spec, subdim=False, uops_sha=uops_sha)
    dve_ops.OPS.append(op)
    dve_ops.CUSTOM_DVE_SPECS[NAME] = spec
    dve_ops._SUB_OPCODE_FOR_NAME[NAME] = row
    return op


def build_nc(npc=NPC, nlist=NLIST, d=D, chunk=CHUNK, n_queues=4):
    """Build and compile the per-core Bass module. Same program on all cores."""
    from contextlib import ExitStack

    import concourse.tile as tile
    from concourse import bacc, bass, mybir

    k1, k2 = d + 2, 2 * d
    nt = npc // P
    assert nt % chunk == 0
    fp32 = mybir.dt.float32
    fp16 = mybir.dt.float16

    argmax_op = _register_argmax_op()

    nc = bacc.Bacc(
        "TRN2",
        target_bir_lowering=False,
        debug=False,
        num_devices=NCORES,
        num_swdge_queues=n_queues,
    )
    vt1 = nc.dram_tensor("vt1", [k1, npc], fp16, kind="ExternalInput").ap()
    vt2 = nc.dram_tensor("vt2", [k2, npc], fp16, kind="ExternalInput").ap()
    ct1 = nc.dram_tensor("ct1", [k1, nlist], fp16, kind="ExternalInput").ap()
    ct2 = nc.dram_tensor("ct2", [k2, nlist], fp16, kind="ExternalInput").ap()
    ctab = nc.dram_tensor("ctab", [nlist, d], fp32, kind="ExternalInput").ap()
    out = nc.dram_tensor("out", [npc, d], fp32, kind="ExternalOutput").ap()

    with tile.TileContext(nc) as tc, ExitStack() as ctx:
        const_pool = ctx.enter_context(tc.tile_pool(name="const", bufs=1))
        vchunk_pool = ctx.enter_context(tc.tile_pool(name="vchunk", bufs=3))
        psum_pool = ctx.enter_context(tc.tile_pool(name="psum", bufs=4, space="PSUM"))
        small_pool = ctx.enter_context(tc.tile_pool(name="small", bufs=4))
        scratch_pool = ctx.enter_context(tc.tile_pool(name="scratch", bufs=2))
        gout_pool = ctx.enter_context(tc.tile_pool(name="gout", bufs=4))

        ct1_sb = const_pool.tile([k1, nlist], fp16)
        nc.sync.dma_start(ct1_sb[:], ct1[:])
        ct2_sb = const_pool.tile([k2, nlist], fp16)
        nc.sync.dma_start(ct2_sb[:], ct2[:])

        for c in range(nt // chunk):
            # strided slab loads of the transposed split vecs
            vch1 = vchunk_pool.tile([k1, chunk * P], fp16, tag="vch1")
            nc.sync.dma_start(vch1[:], vt1[:, c * chunk * P : (c + 1) * chunk * P])
            vch2 = vchunk_pool.tile([k2, chunk * P], fp16, tag="vch2")
            nc.sync.dma_start(vch2[:], vt2[:, c * chunk * P : (c + 1) * chunk * P])
            # process tiles in groups of gb: dense 4-tile matmul bursts keep
            # the PE busy past the HAM warmup window; one index-cast per group.
            # (HW SWDGE honors only one dynamic offset per partition, so the
            # gathers stay per-tile.)
            gb = min(4, chunk)
            for g0 in range(0, chunk, gb):
                pss = []
                for g in range(g0, g0 + gb):
                    w1 = vch1[:, g * P : (g + 1) * P]
                    w2 = vch2[:, g * P : (g + 1) * P]
                    ps = psum_pool.tile([P, nlist], fp32)
                    # same-weight matmuls adjacent (w1,w1,w2,w2)
                    for h in range(nlist // 512):
                        sl = slice(h * 512, (h + 1) * 512)
                        nc.tensor.matmul(
                            ps[:, sl], lhsT=w1, rhs=ct1_sb[:, sl],
                            start=True, stop=False,
                        )
                    for h in range(nlist // 512):
                        sl = slice(h * 512, (h + 1) * 512)
                        nc.tensor.matmul(
                            ps[:, sl], lhsT=w2, rhs=ct2_sb[:, sl],
                            start=False, stop=True,
                        )
                    pss.append(ps)
                idxf = small_pool.tile([P, gb], fp32, tag="idxf")
                for i, ps in enumerate(pss):
                    scr = scratch_pool.tile([P, nlist], fp32, tag="scr")
                    nc.vector._custom_dve(
                        argmax_op, out=scr[:], in0=ps[:],
                        accum_out=idxf[:, i : i + 1],
                    )
                idxu = small_pool.tile([P, gb], mybir.dt.uint32, tag="idxu")
                nc.vector.tensor_copy(idxu[:], idxf[:])
                gout = gout_pool.tile([P, gb * d], fp32)
                for i in range(gb):
                    nc.gpsimd.indirect_dma_start(
                        out=gout[:, i * d : (i + 1) * d],
                        out_offset=None,
                        in_=ctab[:],
                        in_offset=bass.IndirectOffsetOnAxis(ap=idxu[:, i : i + 1], axis=0),
                    )
                # store group: row (c*chunk + g0 + i)*P + p <- gout[p, i*d:(i+1)*d]
                r0 = (c * chunk + g0) * P
                dst = out[r0 : r0 + gb * P, :].rearrange("(g p) d -> p g d", p=P)
                nc.sync.dma_start(dst, gout[:].rearrange("p (g d) -> p g d", d=d))

    nc.compile()
    return nc


def _split16(a):
    hi = a.astype(np.float16)
    lo = (a - hi.astype(np.float32)).astype(np.float16)
    return hi, lo


def _prep_inputs(vecs, centroids):
    """Host-side shard + layout prep. Returns per-core input maps."""
    vecs = np.ascontiguousarray(np.asarray(vecs), dtype=np.float32)
    cents = np.ascontiguousarray(np.asarray(centroids), dtype=np.float32)
    csq = np.sum(cents * cents, axis=1, dtype=np.float32)
    b1, b2 = _split16(-0.5 * csq)
    g1, g2 = _split16(cents)

    ct1 = np.empty((K1, NLIST), dtype=np.float16)
    ct1[:D] = g1.T
    ct1[D] = b1
    ct1[D + 1] = b2
    ct2 = np.empty((K2, NLIST), dtype=np.float16)
    ct2[:D] = g2.T
    ct2[D:] = g1.T

    in_maps = []
    for c in range(NCORES):
        sl = vecs[c * NPC : (c + 1) * NPC]
        h1, h2 = _split16(sl)
        vt1 = np.empty((K1, NPC), dtype=np.float16)
        vt1[:D] = h1.T
        vt1[D:] = 1.0
        vt2 = np.empty((K2, NPC), dtype=np.float16)
        vt2[:D] = h1.T
        vt2[D:] = h2.T
        in_maps.append({"vt1": vt1, "vt2": vt2, "ct1": ct1, "ct2": ct2, "ctab": cents})
    return in_maps


def kernel(vecs, centroids):
    from concourse.bass_utils import run_bass_kernel_spmd

    if "nc" not in _cached:
        _cached["nc"] = build_nc()
    nc = _cached["nc"]

    in_maps = _prep_inputs(vecs, centroids)
    res = run_bass_kernel_spmd(nc, in_maps, core_ids=list(range(NCORES)))
    outs = [res.results[c]["out"] for c in range(NCORES)]
    return np.concatenate(outs, axis=0)
